# revision 32
# baseline (speedup 1.0000x reference)
"""CorrelationHead Trainium2 kernel (parity-class packed, v6).

Math: SpatialCorrelationSampler(patch=16, dil=2) on 7x7 maps -> corr features
are exactly the per-RoI Gram matrix G[b][kl,ij] = sum_c x1[b,c,ij]*x2[b,c,kl],
valid only when (k-i) and (l-j) are both even (parity match). The 49 spatial
positions split into 4 parity classes (i%2,j%2) of sizes 16/12/12/9; valid
(ij,kl) pairs live in the 4 class-diagonal blocks (625 of 2401).

Device: spatial columns host-permuted class-major. Gram per RoI = 4 class
matmuls (K=128 channels x2, M=s_c) written to 32-aligned PSUM partition
blocks (0/32/64/96) via explicit tile_position, each class's output at local
columns 0:s_c so a whole 4-RoI group evicts as ONE partition-0-based DVE
copy into gsaP[row, g, b] (row = ROF[c]+kl_c, +row 105 = ones for bias; g =
ij index within its class). fc1 = 16x2 matmuls of K=128 accumulating
[128b, 1024] PSUM with b1 folded in via the ones row. fc2/fc3 via PE
transpose + K=128 matmuls.

DMA lessons baked in: per-dma_start fixed cost ~2us serialized per HWDGE
ring -> few, large transfers; non-128-partition transfers fall into a slow
2-queue path -> every DMA'd tensor is padded to 128 partitions; small
tensors are packed into one consts transfer. x1 on the SP ring, x2+w1 on
the ACT ring, in 2 RoI-waves so the Gram phase overlaps wave 2. PE warm-up
dummy matmuls lift the HAM clock throttle during the DMA window.

Sharding: pure data-parallel over the 1024 RoIs -> 128 per each of 8 cores;
weights replicated.
"""

import os
import numpy as np

import concourse.bass as bass
import concourse.mybir as mybir
from concourse.bass_utils import run_bass_kernel_spmd

# ---------------------------------------------------------------- constants
P = 16
DIL = 2
H = 7
C = 256
B = 1024
REP = 1024
HW = H * H  # 49
N_CORES = 8
BL = B // N_CORES  # 128 RoIs per core

NG = 4                  # RoIs per PSUM gram group
NGROUP = BL // NG       # 32
PSG_RING = 3
NWAVE = 2               # x DMA waves (RoI halves)
WB = BL // NWAVE        # 64 RoIs per wave

NDUMMY = 80             # PE warm-up matmuls (~8.6us cold)

F32 = mybir.dt.float32
BF16 = mybir.dt.bfloat16

LAST_EXEC_NS = None
_CACHE = {}

# ---------------------------------------------------------------- class map
# class order ee, oo, eo, oe so that (ee+oo) and (eo+oe) are column-adjacent
# and each pair fits one PE col-group tile (25 and 24 cols)
_CLS = [(0, 0), (1, 1), (0, 1), (1, 0)]
PERM = []           # class-major list of original ij = i*7+j
OFF = []            # class start offsets within 0..48 (column space)
SZ = []             # class sizes
for (pi, pj) in _CLS:
    OFF.append(len(PERM))
    n0 = len(PERM)
    for i in range(H):
        if i % 2 != pi:
            continue
        for j in range(H):
            if j % 2 != pj:
                continue
            PERM.append(i * H + j)
    SZ.append(len(PERM) - n0)
assert len(PERM) == HW and SZ == [16, 9, 12, 12]
# two gram tiles: (column range, psum/gsaP row base)
TI = [(0, 25, 0), (25, 49, 32)]
GOF = [0, 16, 32, 44]   # gsaP row offset per class (within its tile block)
GCOL = [0, 16, 0, 12]   # gsaP g-column offset per class
NGRP = 25               # fc1 groups (g columns)
NEV = 56                # rows covered by evictions
BIAS_ROW = 56
KF = 64                 # fc1 contraction rows (padded)

# consts packing: [128, CC] bf16
C_ID = 0                  # idents  [128, 0:128]
C_DUM = 128               # dums    [128, 128:256]
C_W3 = 256                # w3e     [128, 256:288]
C_ONE = 288               # ones    [0,   288:416]
C_B2 = 416                # b2      [0,   416:1440]
C_B3 = 1440               # b3      [0,   1440:1444]
CC = 1444


# ---------------------------------------------------------------- host prep
def _w1p(W1, b1, np_dt):
    """[KF, NGRP, REP]: rows ROF[c]+a hold W1[:, feat(ij,kl)] for class-local
    (kl=a, ij=g); row BIAS_ROW group 0 holds b1; all else zero."""
    w = np.zeros((KF, NGRP, REP), dtype=np.float32)
    for c in range(4):
        off, s = OFF[c], SZ[c]
        for a in range(s):
            kl = PERM[off + a]
            k, l = divmod(kl, H)
            for g in range(s):
                ij = PERM[off + g]
                i, j = divmod(ij, H)
                ph = (k - i) // 2 + 7
                pw = (l - j) // 2 + 7
                f = (ph * P + pw) * HW + ij
                w[GOF[c] + a, GCOL[c] + g, :] = W1[:, f]
    w[BIAS_ROW, 0, :] = b1
    return w.astype(np_dt)


def _ginit(np_dt):
    """gsaP init image: zeros + ones row for bias."""
    g = np.zeros((KF, NGRP, BL), dtype=np.float32)
    g[BIAS_ROW, 0, :] = 1.0
    return g.astype(np_dt)


def _consts(W3, b2, b3, np_dt):
    c = np.zeros((128, CC), dtype=np.float32)
    c[:, C_ID:C_ID + 128] = np.eye(128)
    c[:, C_DUM:C_DUM + 128] = 0.125
    c[:, C_W3:C_W3 + 32] = np.ascontiguousarray(
        W3.T.reshape(8, 128, 4).transpose(1, 0, 2).reshape(128, 32))
    c[0, C_ONE:C_ONE + 128] = 1.0
    c[0, C_B2:C_B2 + REP] = b2
    c[0, C_B3:C_B3 + 4] = b3
    return c.astype(np_dt)


# ---------------------------------------------------------------- device IR
def _build(dt):
    nc = bass.Bass()

    XC = BL * HW   # 6272 columns for all 128 RoIs
    WC = WB * HW   # 3136 columns per wave

    x1h = nc.dram_tensor("x1h", [128, 2, XC], dt, kind="ExternalInput")
    x2h = nc.dram_tensor("x2h", [128, 2, XC], dt, kind="ExternalInput")
    w1h = nc.dram_tensor("w1h", [KF, NGRP * REP], dt, kind="ExternalInput")
    gih = nc.dram_tensor("gih", [KF, NGRP, BL], dt, kind="ExternalInput")
    w2h = nc.dram_tensor("w2h", [128, 8 * REP], dt, kind="ExternalInput")
    ch = nc.dram_tensor("ch", [128, CC], dt, kind="ExternalInput")
    zbh = nc.dram_tensor("zbh", [128, 1], F32, kind="ExternalInput")
    outh = nc.dram_tensor("outh", [128, 4], F32, kind="ExternalOutput")

    from contextlib import ExitStack

    with ExitStack() as ctx:
        sb = lambda name, shape, d: ctx.enter_context(nc.sbuf_tensor(name, shape, d))
        ps = lambda name, shape, d: ctx.enter_context(nc.psum_tensor(name, shape, d))
        sem = lambda name: ctx.enter_context(nc.semaphore(name))

        x1s = sb("x1s", [128, 2, XC], dt)
        x2s = sb("x2s", [128, 2, XC], dt)
        gsaP = sb("gsaP", [KF, NGRP, BL], dt)
        w1r = sb("w1r", [KF, NGRP * REP], dt)
        w2s = sb("w2s", [128, 8 * REP], dt)
        cs = sb("cs", [128, CC], dt)
        zbias = sb("zbias", [128, 1], F32)
        dscr = sb("dscr", [128, 1], dt)
        relu1 = sb("relu1", [128, REP], dt)
        r1T = sb("r1T", [128, REP], dt)
        relu2 = sb("relu2", [128, REP], dt)
        r2T = sb("r2T", [128, REP], dt)
        outs = sb("outs", [128, 4], F32)

        idents = cs[:, C_ID:C_ID + 128]
        dums = cs[:, C_DUM:C_DUM + 128]
        w3s = cs[:, C_W3:C_W3 + 32]
        ones = cs[0:1, C_ONE:C_ONE + 128]
        b2s = cs[0:1, C_B2:C_B2 + REP]
        b3s = cs[0:1, C_B3:C_B3 + 4]

        psG = [ps(f"psG{q}", [NEV, NG, NGRP], F32) for q in range(PSG_RING)]
        psF = [ps(f"psF{h}", [128, 512], F32) for h in range(2)]
        psT = [ps(f"psT{h}", [128, 128], dt) for h in range(2)]
        psO = ps("psO", [128, 4], F32)

        s_w = sem("s_w")
        s_xw = [sem(f"s_xw{w}") for w in range(NWAVE)]
        s_w1a = sem("s_w1a")
        s_ms = sem("s_ms")
        s_g = sem("s_g")
        s_ed = sem("s_ed")
        s_f1 = sem("s_f1")
        s_r1 = sem("s_r1")
        s_t1 = sem("s_t1")
        s_c1 = sem("s_c1")
        s_f2 = sem("s_f2")
        s_r2 = sem("s_r2")
        s_t2 = sem("s_t2")
        s_c2 = sem("s_c2")
        s_f3 = sem("s_f3")
        s_oe = sem("s_oe")
        s_o = sem("s_o")

        block = ctx.enter_context(nc.Block())
        N_WDMA = 3 * 16  # consts, zbias, w2 on SP

        # ---------------- SP: consts + x1 waves + w1 + w2 + output
        @block.sync
        def _(sp):
            sp.dma_start(cs[:, :], ch[:, :]).then_inc(s_w, 16)
            for w in range(NWAVE):
                lo, hi = w * WC, (w + 1) * WC
                sp.dma_start(
                    x1s[:, :, lo:hi], x1h[:, :, lo:hi]
                ).then_inc(s_xw[w], 16)
            sp.dma_start(zbias[:, :], zbh[:, :]).then_inc(s_w, 16)
            sp.dma_start(w2s[:, :], w2h[:, :]).then_inc(s_w, 16)
            sp.wait_ge(s_oe, 1)
            sp.dma_start(outh[:, :], outs[:, :]).then_inc(s_o, 16)
            sp.wait_ge(s_o, 16)

        # ---------------- PE: warm-up, gram, fc1, transposes, fc2, fc3
        @block.tensor
        def _(pe):
            pe.wait_ge(s_w, 16)
            for _ in range(NDUMMY):
                pe.matmul(psF[0][:, 0:128], dums, dums, start=True, stop=True)

            # Gram phase: 4 class-diagonal blocks per RoI at 32-aligned rows,
            # outputs at LOCAL columns 0:s so each group evicts as one copy
            pe.wait_ge(s_ms, PSG_RING)
            for gi in range(NGROUP):
                if gi % (NGROUP // NWAVE) == 0:
                    pe.wait_ge(s_xw[gi // (NGROUP // NWAVE)], 32)
                q = gi % PSG_RING
                if gi >= PSG_RING:
                    pe.wait_ge(s_ed, gi - PSG_RING + 1)
                for bb in range(NG):
                    base = (gi * NG + bb) * HW
                    # tile-outer, half-inner: start=True clears has_written
                    # for the whole bank, so each tile's accumulation pair
                    # must finish before the next tile starts
                    for (lo, hi, rb) in TI:
                        s = hi - lo
                        for t in range(2):
                            mm = pe.matmul(
                                psG[q][rb:rb + s, bb, 0:s],
                                x2s[:, t, base + lo:base + hi],
                                x1s[:, t, base + lo:base + hi],
                                start=(t == 0),
                                stop=(t == 1),
                                tile_position=(0, rb),
                            )
                mm.then_inc(s_g, 1)

            # fc1: 16 group matmuls x 2 halves, bias via ones row
            pe.wait_ge(s_ed, NGROUP)
            pe.wait_ge(s_w1a, 32)
            for g in range(NGRP):
                for hf in range(2):
                    mm = pe.matmul(
                        psF[hf][:, :],
                        gsaP[:, g, :],
                        w1r[:, g * REP + hf * 512:g * REP + hf * 512 + 512],
                        start=(g == 0),
                        stop=(g == NGRP - 1),
                    )
                    if g == NGRP - 1:
                        mm.then_inc(s_f1, 1)

            # transpose relu1 -> r1T
            pe.wait_ge(s_w, N_WDMA)
            for k in range(8):
                pe.wait_ge(s_r1, 1 if k < 4 else 2)
                if k >= 2:
                    pe.wait_ge(s_c1, k - 1)
                pe.transpose(
                    psT[k % 2][:, :], relu1[:, k * 128:(k + 1) * 128], idents
                ).then_inc(s_t1, 1)

            # fc2
            for k in range(8):
                pe.wait_ge(s_c1, k + 1)
                for hf in range(2):
                    pe.matmul(
                        psF[hf][:, :],
                        r1T[:, k * 128:(k + 1) * 128],
                        w2s[:, k * REP + hf * 512:k * REP + hf * 512 + 512],
                        start=(k == 0),
                        stop=False,
                    )
            for hf in range(2):
                pe.matmul(
                    psF[hf][:, :], ones, b2s[:, hf * 512:hf * 512 + 512],
                    start=False, stop=True,
                ).then_inc(s_f2, 1)

            # transpose relu2 -> r2T
            for k in range(8):
                pe.wait_ge(s_r2, 1 if k < 4 else 2)
                if k >= 2:
                    pe.wait_ge(s_c2, k - 1)
                pe.transpose(
                    psT[k % 2][:, :], relu2[:, k * 128:(k + 1) * 128], idents
                ).then_inc(s_t2, 1)

            # fc3
            for k in range(8):
                pe.wait_ge(s_c2, k + 1)
                pe.matmul(
                    psO[:, :],
                    r2T[:, k * 128:(k + 1) * 128],
                    w3s[:, k * 4:(k + 1) * 4],
                    start=(k == 0),
                    stop=False,
                )
            pe.matmul(
                psO[:, :], ones, b3s, start=False, stop=True
            ).then_inc(s_f3, 1)

        # ---------------- ACT: gih + x2 waves + w1 on its own HWDGE ring;
        # engine stays idle through the gram phase so generation isn't starved
        @block.scalar
        def _(act):
            act.dma_start(gsaP[:, :, :], gih[:, :, :]).then_inc(s_w1a, 16)
            for w in range(NWAVE):
                lo, hi = w * WC, (w + 1) * WC
                act.dma_start(
                    x2s[:, :, lo:hi], x2h[:, :, lo:hi]
                ).then_inc(s_xw[w], 16)
            act.dma_start(w1r[:, :], w1h[:, :]).then_inc(s_w1a, 16)
            # warm the activation table while idle
            act.wait_ge(s_w, 16)
            act.activation(dscr[:, :], dums[:, 0:1],
                           mybir.ActivationFunctionType.Copy)
            act.wait_ge(s_w, N_WDMA)
            for hf in range(2):
                act.wait_ge(s_f1, hf + 1)
                act.activation(
                    relu1[:, hf * 512:(hf + 1) * 512], psF[hf][:, :],
                    mybir.ActivationFunctionType.Relu, bias=zbias[:, :],
                ).then_inc(s_r1, 1)
            for hf in range(2):
                act.wait_ge(s_f2, hf + 1)
                act.activation(
                    relu2[:, hf * 512:(hf + 1) * 512], psF[hf][:, :],
                    mybir.ActivationFunctionType.Relu, bias=zbias[:, :],
                ).then_inc(s_r2, 1)
            act.wait_ge(s_f3, 1)
            act.activation(
                outs[:, :], psO[:, :], mybir.ActivationFunctionType.Copy
            ).then_inc(s_oe, 1)

        # ---------------- DVE: psG pre-zero, fused evictions, copybacks
        @block.vector
        def _(dve):
            for q in range(PSG_RING):
                dve.memset(psG[q][:, :, :], 0.0).then_inc(s_ms, 1)
            dve.wait_ge(s_w1a, 16)
            for gi in range(NGROUP):
                q = gi % PSG_RING
                dve.wait_ge(s_g, gi + 1)
                dve.tensor_copy(
                    gsaP[0:NEV, :, gi * NG:(gi + 1) * NG],
                    psG[q][:, :, :].rearrange("p b i -> p i b"),
                ).then_inc(s_ed, 1)
            for k in range(8):
                dve.wait_ge(s_t1, k + 1)
                dve.tensor_copy(
                    r1T[:, k * 128:(k + 1) * 128], psT[k % 2][:, :]
                ).then_inc(s_c1, 1)
            for k in range(8):
                dve.wait_ge(s_t2, k + 1)
                dve.tensor_copy(
                    r2T[:, k * 128:(k + 1) * 128], psT[k % 2][:, :]
                ).then_inc(s_c2, 1)

    return nc


def _get_nc(dt):
    key = ("nc", str(dt))
    if key not in _CACHE:
        _CACHE[key] = _build(dt)
    return _CACHE[key]


# ---------------------------------------------------------------- entry
def kernel(patch1, patch2, W1, b1, W2, b2, W3, b3):
    global LAST_EXEC_NS
    import ml_dtypes
    np_dt = ml_dtypes.bfloat16
    dt = BF16

    patch1 = np.asarray(patch1, dtype=np.float32).reshape(B, C, HW)
    patch2 = np.asarray(patch2, dtype=np.float32).reshape(B, C, HW)
    W1 = np.asarray(W1, dtype=np.float32)
    W2 = np.asarray(W2, dtype=np.float32)
    W3 = np.asarray(W3, dtype=np.float32)
    b1 = np.asarray(b1, dtype=np.float32)
    b2 = np.asarray(b2, dtype=np.float32)
    b3 = np.asarray(b3, dtype=np.float32)

    # class-major spatial permutation
    p1 = patch1[:, :, PERM]
    p2 = patch2[:, :, PERM]

    w1p = _w1p(W1, b1, np_dt).reshape(KF, NGRP * REP)
    gini = _ginit(np_dt)
    w2e = np.ascontiguousarray(
        W2.T.reshape(8, 128, REP).transpose(1, 0, 2).reshape(128, 8 * REP)
    ).astype(np_dt)
    consts = _consts(W3, b2, b3, np_dt)

    shared = {
        "w1h": np.ascontiguousarray(w1p),
        "gih": np.ascontiguousarray(gini),
        "w2h": w2e,
        "ch": consts,
        "zbh": np.zeros((128, 1), dtype=np.float32),
    }

    in_maps = []
    for i in range(N_CORES):
        sl = slice(i * BL, (i + 1) * BL)
        # [128 part, 2 half, BL*HW], channel p of half t = channel t*128+p
        x1 = np.ascontiguousarray(
            p1[sl].transpose(1, 0, 2).reshape(2, 128, BL * HW).transpose(1, 0, 2)
        ).astype(np_dt)
        x2 = np.ascontiguousarray(
            p2[sl].transpose(1, 0, 2).reshape(2, 128, BL * HW).transpose(1, 0, 2)
        ).astype(np_dt)
        in_maps.append({
            "x1h": x1, "x2h": x2,
            **{k: np.array(v) for k, v in shared.items()},
        })

    nc = _get_nc(dt)
    trace = os.environ.get("CORR_TRACE", "0") == "1"
    res = run_bass_kernel_spmd(nc, in_maps, list(range(N_CORES)), trace=trace)
    LAST_EXEC_NS = res.exec_time_ns

    out = np.concatenate(
        [res.results[i]["outh"] for i in range(N_CORES)], axis=0
    ).astype(np.float32)
    return out
